# revision 27
# baseline (speedup 1.0000x reference)
"""Trainium2 Bass kernel for nn_CaslsChineseAttnLoss (label-smoothed KLDiv loss).

Math (per flattened token n, vocab size V):
    weight row = off_n everywhere except src_n at the target column t_n, with
        off_n = sm_n * matric[forth_n, t_n] / (V-1),  src_n = 1 - V*off_n
    kl_n = (V-1)*off*ln(off) + src*ln(src) - off*S_n - (src-off)*logp_{n,t_n}
    where S_n = sum_v logp_{n,v} = sumx_n - V*lse_n, lse_n = ln(sum_v exp x_nv).
    loss = sum_n kl_n / sum_b (label_lengths_b + 1)

Sharding: data-parallel over the token dim N=4096 — 512 rows per core across
8 cores; matric replicated (device-side indirect-DMA gather of the 512
confusion values per core); each core emits its partial sum and the host
combines the 8 partials (an on-device AllReduce psum was measured at ~30us
of cross-core skew-wait for a 4-byte payload, dwarfing the 8-float host add).

Device kernel per core: stream the [512, 8192] f32 shard through SBUF in
[128, 2048] chunks; ACT computes exp with accum (row sum-exp), DVE reduces
row sums; both overlap under the HBM DMA.  exp is computed without max
subtraction — inputs are unit-normal logits, so sum-exp stays in fp32 range.
"""

import math

import numpy as np

import concourse.bass as bass
import concourse.tile as tile
from concourse import bacc, mybir
from concourse import bass_utils
from concourse.hw_specs import get_activation_tables

ALPHA = 0.1
B, T, V = 8, 512, 8192
N = B * T                 # 4096 flattened tokens
N_CORES = 8
NLOC = N // N_CORES       # 512 rows per core
P = 128                   # partitions
NT = NLOC // P            # 4 row tiles per core
NCH = 2                   # column chunks per row tile
CHUNK = V // NCH          # 2048
F32 = mybir.dt.float32
I32 = mybir.dt.int32

_CACHE = {}


def _build():
    if "nc" in _CACHE:
        return _CACHE["nc"]

    nc = bacc.Bacc("TRN2", target_bir_lowering=False, debug=False,
                   num_devices=N_CORES)

    x_d = nc.dram_tensor("x", [NLOC, V], F32, kind="ExternalInput")
    mat_d = nc.dram_tensor("mat", [V * V, 1], F32, kind="ExternalInput")
    midx_d = nc.dram_tensor("midx", [P, NT], I32, kind="ExternalInput")
    xgidx_d = nc.dram_tensor("xgidx", [P, NT], I32, kind="ExternalInput")
    lenrow_d = nc.dram_tensor("lenrow", [P, NT], F32, kind="ExternalInput")
    out_d = nc.dram_tensor("out", [1, 1], F32, kind="ExternalOutput")

    AF = mybir.ActivationFunctionType
    AX = mybir.AxisListType.X
    MUL = mybir.AluOpType.mult
    ADD = mybir.AluOpType.add

    # chunk plan: tiles 0-2 stream in NCH equal column chunks; the last tile
    # tapers so the final exp/reduce after the last DMA byte is short
    chunk_plan = []  # (row_tile, col_start, width, part_col)
    pc = 0
    for j in range(NT - 1):
        for k in range(NCH):
            chunk_plan.append((j, k * CHUNK, CHUNK, pc)); pc += 1
    tail_widths = [CHUNK, CHUNK // 2, CHUNK // 4, CHUNK // 8, CHUNK // 8]
    if NCH > 2:
        tail_widths = [CHUNK] * (NCH - 2) + tail_widths
    cs = 0
    for w in tail_widths:
        chunk_plan.append((NT - 1, cs, w, pc)); cs += w; pc += 1
    assert cs == V
    NPARTS = pc
    NP0 = (NT - 1) * NCH  # part cols belonging to tiles 0..NT-2

    with tile.TileContext(nc) as tc:
        with tc.tile_pool(name="xchunk", bufs=5) as xpool, \
             tc.tile_pool(name="scratch", bufs=2) as spool, \
             tc.tile_pool(name="vscratch", bufs=2) as vpool, \
             tc.tile_pool(name="stats", bufs=1) as stats, \
             tc.tile_pool(name="psum", bufs=1, space="PSUM") as psump:

            # pre-load the ACT table set that has BOTH exp and ln, so the
            # greedy per-func table pass inserts zero switches
            tabs = list(get_activation_tables(nc.m.arch).keys())
            nc.scalar.add_instruction(mybir.InstLoadActFuncSet(
                name=nc.get_next_instruction_name(),
                act_func_set_id=tabs.index("natural_log_exp_and_others"),
                ins=[], outs=[]))

            sumexp_parts = stats.tile([P, NPARTS], F32)
            sumx_parts = stats.tile([P, NPARTS], F32)
            midx_sb = stats.tile([P, NT], I32)
            xgidx_sb = stats.tile([P, NT], I32)
            lenr = stats.tile([P, NT], F32)
            ns = stats.tile([P, NT], F32)
            xt = stats.tile([P, NT], F32)
            eps = stats.tile([P, 1], F32)
            nc.vector.memset(eps[:], 1e-30)
            ones = stats.tile([P, 1], F32)
            nc.vector.memset(ones[:], 1.0)
            invlen = stats.tile([P, NT], F32)
            e1 = stats.tile([P, NT], F32)
            smc = stats.tile([P, NT], F32)
            x_flat = bass.AP(tensor=x_d, offset=0, ap=[[1, NLOC * V], [1, 1]])

            def emit_side_loads():
                # idx loads + element gathers + sm-coefficient chain;
                # emitted after chunk 1 so the x stream owns the ring head
                nc.sync.dma_start(midx_sb[:], midx_d.ap())
                nc.sync.dma_start(xgidx_sb[:], xgidx_d.ap())
                nc.sync.dma_start(lenr[:], lenrow_d.ap())
                for j in range(NT):
                    nc.gpsimd.indirect_dma_start(
                        out=ns[:, j:j + 1], out_offset=None,
                        in_=mat_d.ap(),
                        in_offset=bass.IndirectOffsetOnAxis(
                            ap=midx_sb[:, j:j + 1], axis=0))
                    nc.gpsimd.indirect_dma_start(
                        out=xt[:, j:j + 1], out_offset=None,
                        in_=x_flat,
                        in_offset=bass.IndirectOffsetOnAxis(
                            ap=xgidx_sb[:, j:j + 1], axis=0))
                nc.vector.reciprocal(invlen[:], lenr[:])
                nc.scalar.activation(e1[:], invlen[:], AF.Exp,
                                     scale=math.log(1.0 - ALPHA))
                nc.vector.tensor_scalar(smc[:], e1[:],
                                        -1.0 / (V - 1), 1.0 / (V - 1),
                                        op0=MUL, op1=ADD)

            # per-row constants, folded so the post-stream tail is minimal:
            #   kl_row = c1p - off*sumx + c3*lse        (proof: expand
            #   (V-1)xlogy(off) + xlogy(src) - off*(sumx - V*lse)
            #     - (src-off)*(xt - lse)  with c2 = src-off)
            off = stats.tile([P, NT], F32)
            src = stats.tile([P, NT], F32)
            lnoff = stats.tile([P, NT], F32)
            lnsrc = stats.tile([P, NT], F32)
            c2 = stats.tile([P, NT], F32)
            c3 = stats.tile([P, NT], F32)
            c1p = stats.tile([P, NT], F32)
            tmp = stats.tile([P, NT], F32)

            def emit_const_stats(pin_after):
                i0 = nc.vector.tensor_mul(off[:], smc[:], ns[:])
                # pin the chain root behind a late chunk op: the scheduler's
                # model thinks the gathers land early and would otherwise
                # hoist this chain right after chunk 0, head-blocking both
                # engine streams on the gather semaphore for ~9us
                tile.add_dep_helper(i0.ins, pin_after.ins, False,
                                    "const-stats after mid-stream")
                nc.vector.tensor_scalar(src[:], off[:], -float(V), 1.0,
                                        op0=MUL, op1=ADD)
                nc.scalar.activation(lnoff[:], off[:], AF.Ln, bias=eps[:])
                nc.scalar.activation(lnsrc[:], src[:], AF.Ln)
                nc.vector.tensor_mul(c1p[:], off[:], lnoff[:])
                nc.vector.tensor_scalar(c1p[:], c1p[:], float(V - 1), None,
                                        op0=MUL)
                nc.vector.tensor_mul(tmp[:], src[:], lnsrc[:])
                nc.vector.tensor_add(c1p[:], c1p[:], tmp[:])
                nc.vector.tensor_sub(c2[:], src[:], off[:])
                nc.vector.tensor_scalar(c3[:], off[:], float(V), None,
                                        op0=MUL)
                nc.vector.tensor_add(c3[:], c3[:], c2[:])
                nc.vector.tensor_mul(tmp[:], c2[:], xt[:])
                nc.vector.tensor_sub(c1p[:], c1p[:], tmp[:])

            # streaming pass: per chunk, ACT exp+accum and DVE row-sum
            for ci, (j, c0, w, col) in enumerate(chunk_plan):
                xtile = xpool.tile([P, w], F32, tag="xchunk")
                nc.sync.dma_start(
                    xtile[:], x_d.ap()[j * P:(j + 1) * P, c0:c0 + w])
                sc = spool.tile([P, w], F32, tag="scratch")
                nc.scalar.activation(
                    sc[:], xtile[:], AF.Exp,
                    accum_out=sumexp_parts[:, col:col + 1])
                # row-sum via tensor_scalar+accum: single-src f32 SBUF runs
                # in the 2x DVE perf mode, unlike tensor_reduce (1x only)
                vs = vpool.tile([P, w], F32, tag="vscratch")
                red = nc.vector.tensor_scalar(
                    vs[:], xtile[:], 1.0, 0.0, op0=MUL, op1=ADD,
                    accum_out=sumx_parts[:, col:col + 1])
                if ci == 1:
                    emit_side_loads()
                if ci == 10:
                    emit_const_stats(pin_after=red)

            # scheduler-only fence: keep the tail chain out of the stream
            tc.no_sync_barrier()

            # combine chunk partials (sumx first: off*sumx can then overlap
            # the Ln on ACT)
            sumexp = stats.tile([P, NT], F32)
            sumx = stats.tile([P, NT], F32)
            nc.vector.reduce_sum(
                sumx[:, 0:NT - 1],
                sumx_parts[:, 0:NP0].rearrange("p (j k) -> p j k", k=NCH),
                axis=AX)
            nc.vector.reduce_sum(
                sumx[:, NT - 1:NT], sumx_parts[:, NP0:NPARTS], axis=AX)
            nc.vector.reduce_sum(
                sumexp[:, 0:NT - 1],
                sumexp_parts[:, 0:NP0].rearrange("p (j k) -> p j k", k=NCH),
                axis=AX)
            nc.vector.reduce_sum(
                sumexp[:, NT - 1:NT], sumexp_parts[:, NP0:NPARTS], axis=AX)

            lse = stats.tile([P, NT], F32)
            nc.scalar.activation(lse[:], sumexp[:], AF.Ln)
            acc = stats.tile([P, NT], F32)
            nc.vector.tensor_mul(acc[:], off[:], sumx[:])      # off*sumx
            nc.vector.tensor_sub(acc[:], c1p[:], acc[:])       # c1p - off*sumx
            nc.vector.tensor_mul(tmp[:], c3[:], lse[:])        # c3*lse
            nc.vector.tensor_add(acc[:], acc[:], tmp[:])       # kl rows

            rowsum = stats.tile([P, 1], F32)
            nc.vector.reduce_sum(rowsum[:], acc[:], axis=AX)
            tot_psum = psump.tile([1, 1], F32)
            nc.tensor.matmul(tot_psum[:], lhsT=rowsum[:], rhs=ones[:],
                             start=True, stop=True)
            tot = stats.tile([1, 1], F32)
            nc.scalar.copy(tot[:], tot_psum[:])
            # per-core partial sum; host combines the 8 partials (the
            # cross-core psum via AllReduce costs ~30us of skew-wait, far
            # more than the 8-float host add)
            nc.sync.dma_start(out_d.ap(), tot[:])

    nc.compile()
    _CACHE["nc"] = nc
    return nc


def _prep_in_maps(inputs, matric, targets, label_lengths):
    x = np.ascontiguousarray(np.asarray(inputs, dtype=np.float32)).reshape(N, V)
    t = np.asarray(targets).reshape(-1).astype(np.int64)
    lab = np.asarray(label_lengths).reshape(-1).astype(np.int64)
    mat = np.ascontiguousarray(np.asarray(matric, dtype=np.float32)).reshape(V * V, 1)

    eos = (t == 1)
    prev = np.roll(t, 1)
    is_start = np.roll(eos, 1)
    is_start[0] = True
    forth = np.where(is_start, N - 1, prev)
    seg = np.cumsum(eos.astype(np.int64)) - eos.astype(np.int64)
    length = lab + 1
    # jax gather clamps out-of-range indices; mirror that
    len_row = length[np.clip(seg, 0, B - 1)].astype(np.float32)
    midx = (np.clip(forth, 0, V - 1) * V + np.clip(t, 0, V - 1)).astype(np.int32)
    t_cl = np.clip(t, 0, V - 1)
    lensum = np.float32(length.sum())

    in_maps = []
    for c in range(N_CORES):
        sl = slice(c * NLOC, (c + 1) * NLOC)
        rows = np.arange(NLOC, dtype=np.int64)
        xg = (rows * V + t_cl[sl]).astype(np.int32)
        in_maps.append({
            "x": np.ascontiguousarray(x[sl]),
            "mat": mat,
            "midx": np.ascontiguousarray(midx[sl].reshape(NT, P).T),
            "xgidx": np.ascontiguousarray(xg.reshape(NT, P).T),
            "lenrow": np.ascontiguousarray(
                len_row[sl].reshape(NT, P).T),
        })
    return in_maps, lensum


def run(inputs, matric, targets, label_lengths, trace=False):
    nc = _build()
    in_maps, lensum = _prep_in_maps(inputs, matric, targets, label_lengths)
    if trace:
        _install_ntff_hook()
    res = bass_utils.run_bass_kernel_spmd(
        nc, in_maps, core_ids=list(range(N_CORES)), trace=trace)
    partials = np.array(
        [res.results[c]["out"][0, 0] for c in range(N_CORES)], dtype=np.float32)
    out = np.float32(partials.sum(dtype=np.float32) / lensum)
    return np.asarray(out), res


def kernel(inputs, matric, targets, label_lengths):
    out, _ = run(inputs, matric, targets, label_lengths, trace=False)
    return out


def _install_ntff_hook():
    """bass_utils expects antenv.axon_hooks for NTFF tracing under axon; the
    agent image lacks it, so recreate the ctypes shim inline."""
    import contextlib
    import ctypes
    import sys
    import types

    if "antenv.axon_hooks" in sys.modules:
        return
    so_path = "/opt/axon/libaxon_pjrt.so"
    try:
        lib = ctypes.CDLL(so_path)
    except OSError:
        return
    if not hasattr(lib, "axon_start_nrt_profile"):
        return
    lib.axon_start_nrt_profile.argtypes = [
        ctypes.POINTER(ctypes.c_int64), ctypes.c_size_t]
    lib.axon_start_nrt_profile.restype = ctypes.c_int64
    lib.axon_stop_nrt_profile.argtypes = [ctypes.c_char_p]
    lib.axon_stop_nrt_profile.restype = ctypes.c_int64

    @contextlib.contextmanager
    def _hook(output_dir, device_ids):
        import jax
        jax.devices()
        ids = list(device_ids) if device_ids else []
        arr = (ctypes.c_int64 * len(ids))(*ids)
        rc = lib.axon_start_nrt_profile(arr, len(ids))
        if rc != 0:
            raise RuntimeError(f"axon_start_nrt_profile rc={rc}")
        try:
            yield
        finally:
            n = lib.axon_stop_nrt_profile(str(output_dir).encode())
            if n < 0:
                raise RuntimeError(f"axon_stop_nrt_profile rc={n}")

    mod = types.ModuleType("antenv.axon_hooks")
    mod.get_axon_ntff_profile_hook = lambda: _hook
    mod.set_axon_ntff_profile_hook = lambda h: None
    sys.modules["antenv.axon_hooks"] = mod


# revision 28
# speedup vs baseline: 1.1132x; 1.1132x over previous
"""Trainium2 Bass kernel for nn_CaslsChineseAttnLoss (label-smoothed KLDiv loss).

Math (per flattened token n, vocab size V):
    weight row = off_n everywhere except src_n at the target column t_n, with
        off_n = sm_n * matric[forth_n, t_n] / (V-1),  src_n = 1 - V*off_n
    kl_n = (V-1)*off*ln(off) + src*ln(src) - off*S_n - (src-off)*logp_{n,t_n}
    where S_n = sum_v logp_{n,v} = sumx_n - V*lse_n, lse_n = ln(sum_v exp x_nv).
    loss = sum_n kl_n / sum_b (label_lengths_b + 1)

Sharding: data-parallel over the token dim N=4096 — 512 rows per core across
8 cores; matric replicated (device-side indirect-DMA gather of the 512
confusion values per core); each core emits its partial sum and the host
combines the 8 partials (an on-device AllReduce psum was measured at ~30us
of cross-core skew-wait for a 4-byte payload, dwarfing the 8-float host add).

Device kernel per core: stream the [512, 8192] f32 shard through SBUF in
[128, 2048] chunks; ACT computes exp with accum (row sum-exp), DVE reduces
row sums; both overlap under the HBM DMA.  exp is computed without max
subtraction — inputs are unit-normal logits, so sum-exp stays in fp32 range.
"""

import math

import numpy as np

import concourse.bass as bass
import concourse.tile as tile
from concourse import bacc, mybir
from concourse import bass_utils
from concourse.hw_specs import get_activation_tables

ALPHA = 0.1
B, T, V = 8, 512, 8192
N = B * T                 # 4096 flattened tokens
N_CORES = 8
NLOC = N // N_CORES       # 512 rows per core
P = 128                   # partitions
NT = NLOC // P            # 4 row tiles per core
NCH = 4                   # column chunks per row tile
CHUNK = V // NCH          # 2048
F32 = mybir.dt.float32
I32 = mybir.dt.int32

_CACHE = {}


def _build():
    if "nc" in _CACHE:
        return _CACHE["nc"]

    nc = bacc.Bacc("TRN2", target_bir_lowering=False, debug=False,
                   num_devices=N_CORES)

    x_d = nc.dram_tensor("x", [NLOC, V], F32, kind="ExternalInput")
    mat_d = nc.dram_tensor("mat", [V * V, 1], F32, kind="ExternalInput")
    midx_d = nc.dram_tensor("midx", [P, NT], I32, kind="ExternalInput")
    xgidx_d = nc.dram_tensor("xgidx", [P, NT], I32, kind="ExternalInput")
    lenrow_d = nc.dram_tensor("lenrow", [P, NT], F32, kind="ExternalInput")
    out_d = nc.dram_tensor("out", [1, 1], F32, kind="ExternalOutput")

    AF = mybir.ActivationFunctionType
    AX = mybir.AxisListType.X
    MUL = mybir.AluOpType.mult
    ADD = mybir.AluOpType.add

    # chunk plan: tiles 0-2 stream in NCH equal column chunks; the last tile
    # tapers so the final exp/reduce after the last DMA byte is short
    chunk_plan = []  # (row_tile, col_start, width, part_col)
    pc = 0
    for j in range(NT - 1):
        for k in range(NCH):
            chunk_plan.append((j, k * CHUNK, CHUNK, pc)); pc += 1
    tail_widths = [CHUNK, CHUNK // 2, CHUNK // 4, CHUNK // 8, CHUNK // 8]
    if NCH > 2:
        tail_widths = [CHUNK] * (NCH - 2) + tail_widths
    cs = 0
    for w in tail_widths:
        chunk_plan.append((NT - 1, cs, w, pc)); cs += w; pc += 1
    assert cs == V
    NPARTS = pc
    NP0 = (NT - 1) * NCH  # part cols belonging to tiles 0..NT-2

    with tile.TileContext(nc) as tc:
        with tc.tile_pool(name="xchunk", bufs=10) as xpool, \
             tc.tile_pool(name="scratch", bufs=2) as spool, \
             tc.tile_pool(name="vscratch", bufs=2) as vpool, \
             tc.tile_pool(name="stats", bufs=1) as stats, \
             tc.tile_pool(name="psum", bufs=1, space="PSUM") as psump:

            # pre-load the ACT table set that has BOTH exp and ln, so the
            # greedy per-func table pass inserts zero switches
            tabs = list(get_activation_tables(nc.m.arch).keys())
            nc.scalar.add_instruction(mybir.InstLoadActFuncSet(
                name=nc.get_next_instruction_name(),
                act_func_set_id=tabs.index("natural_log_exp_and_others"),
                ins=[], outs=[]))

            sumexp_parts = stats.tile([P, NPARTS], F32)
            sumx_parts = stats.tile([P, NPARTS], F32)
            midx_sb = stats.tile([P, NT], I32)
            xgidx_sb = stats.tile([P, NT], I32)
            lenr = stats.tile([P, NT], F32)
            ns = stats.tile([P, NT], F32)
            xt = stats.tile([P, NT], F32)
            eps = stats.tile([P, 1], F32)
            nc.vector.memset(eps[:], 1e-30)
            ones = stats.tile([P, 1], F32)
            nc.vector.memset(ones[:], 1.0)
            invlen = stats.tile([P, NT], F32)
            e1 = stats.tile([P, NT], F32)
            smc = stats.tile([P, NT], F32)
            x_flat = bass.AP(tensor=x_d, offset=0, ap=[[1, NLOC * V], [1, 1]])

            def emit_side_loads():
                # idx loads + element gathers + sm-coefficient chain;
                # emitted after chunk 1 so the x stream owns the ring head
                nc.sync.dma_start(midx_sb[:], midx_d.ap())
                nc.sync.dma_start(xgidx_sb[:], xgidx_d.ap())
                nc.sync.dma_start(lenr[:], lenrow_d.ap())
                for j in range(NT):
                    nc.gpsimd.indirect_dma_start(
                        out=ns[:, j:j + 1], out_offset=None,
                        in_=mat_d.ap(),
                        in_offset=bass.IndirectOffsetOnAxis(
                            ap=midx_sb[:, j:j + 1], axis=0))
                    nc.gpsimd.indirect_dma_start(
                        out=xt[:, j:j + 1], out_offset=None,
                        in_=x_flat,
                        in_offset=bass.IndirectOffsetOnAxis(
                            ap=xgidx_sb[:, j:j + 1], axis=0))
                nc.vector.reciprocal(invlen[:], lenr[:])
                nc.scalar.activation(e1[:], invlen[:], AF.Exp,
                                     scale=math.log(1.0 - ALPHA))
                nc.vector.tensor_scalar(smc[:], e1[:],
                                        -1.0 / (V - 1), 1.0 / (V - 1),
                                        op0=MUL, op1=ADD)

            # per-row constants, folded so the post-stream tail is minimal:
            #   kl_row = c1p - off*sumx + c3*lse        (proof: expand
            #   (V-1)xlogy(off) + xlogy(src) - off*(sumx - V*lse)
            #     - (src-off)*(xt - lse)  with c2 = src-off)
            off = stats.tile([P, NT], F32)
            src = stats.tile([P, NT], F32)
            lnoff = stats.tile([P, NT], F32)
            lnsrc = stats.tile([P, NT], F32)
            c2 = stats.tile([P, NT], F32)
            c3 = stats.tile([P, NT], F32)
            c1p = stats.tile([P, NT], F32)
            tmp = stats.tile([P, NT], F32)

            def emit_const_stats(pin_after):
                i0 = nc.vector.tensor_mul(off[:], smc[:], ns[:])
                # pin the chain root behind a late chunk op: the scheduler's
                # model thinks the gathers land early and would otherwise
                # hoist this chain right after chunk 0, head-blocking both
                # engine streams on the gather semaphore for ~9us
                tile.add_dep_helper(i0.ins, pin_after.ins, False,
                                    "const-stats after mid-stream")
                nc.vector.tensor_scalar(src[:], off[:], -float(V), 1.0,
                                        op0=MUL, op1=ADD)
                nc.scalar.activation(lnoff[:], off[:], AF.Ln, bias=eps[:])
                nc.scalar.activation(lnsrc[:], src[:], AF.Ln)
                nc.vector.tensor_mul(c1p[:], off[:], lnoff[:])
                nc.vector.tensor_scalar(c1p[:], c1p[:], float(V - 1), None,
                                        op0=MUL)
                nc.vector.tensor_mul(tmp[:], src[:], lnsrc[:])
                nc.vector.tensor_add(c1p[:], c1p[:], tmp[:])
                nc.vector.tensor_sub(c2[:], src[:], off[:])
                nc.vector.tensor_scalar(c3[:], off[:], float(V), None,
                                        op0=MUL)
                nc.vector.tensor_add(c3[:], c3[:], c2[:])
                nc.vector.tensor_mul(tmp[:], c2[:], xt[:])
                nc.vector.tensor_sub(c1p[:], c1p[:], tmp[:])

            # streaming pass: per chunk, ACT exp+accum and DVE row-sum
            for ci, (j, c0, w, col) in enumerate(chunk_plan):
                xtile = xpool.tile([P, w], F32, tag="xchunk")
                nc.sync.dma_start(
                    xtile[:], x_d.ap()[j * P:(j + 1) * P, c0:c0 + w])
                sc = spool.tile([P, w], F32, tag="scratch")
                nc.scalar.activation(
                    sc[:], xtile[:], AF.Exp,
                    accum_out=sumexp_parts[:, col:col + 1])
                # row-sum via tensor_scalar+accum: single-src f32 SBUF runs
                # in the 2x DVE perf mode, unlike tensor_reduce (1x only)
                vs = vpool.tile([P, w], F32, tag="vscratch")
                red = nc.vector.tensor_scalar(
                    vs[:], xtile[:], 1.0, 0.0, op0=MUL, op1=ADD,
                    accum_out=sumx_parts[:, col:col + 1])
                if ci == 1:
                    emit_side_loads()
                if ci == 10:
                    emit_const_stats(pin_after=red)

            # scheduler-only fence: keep the tail chain out of the stream
            tc.no_sync_barrier()

            # combine chunk partials (sumx first: off*sumx can then overlap
            # the Ln on ACT)
            sumexp = stats.tile([P, NT], F32)
            sumx = stats.tile([P, NT], F32)
            nc.vector.reduce_sum(
                sumx[:, 0:NT - 1],
                sumx_parts[:, 0:NP0].rearrange("p (j k) -> p j k", k=NCH),
                axis=AX)
            nc.vector.reduce_sum(
                sumx[:, NT - 1:NT], sumx_parts[:, NP0:NPARTS], axis=AX)
            nc.vector.reduce_sum(
                sumexp[:, 0:NT - 1],
                sumexp_parts[:, 0:NP0].rearrange("p (j k) -> p j k", k=NCH),
                axis=AX)
            nc.vector.reduce_sum(
                sumexp[:, NT - 1:NT], sumexp_parts[:, NP0:NPARTS], axis=AX)

            lse = stats.tile([P, NT], F32)
            nc.scalar.activation(lse[:], sumexp[:], AF.Ln)
            acc = stats.tile([P, NT], F32)
            nc.vector.tensor_mul(acc[:], off[:], sumx[:])      # off*sumx
            nc.vector.tensor_sub(acc[:], c1p[:], acc[:])       # c1p - off*sumx
            nc.vector.tensor_mul(tmp[:], c3[:], lse[:])        # c3*lse
            nc.vector.tensor_add(acc[:], acc[:], tmp[:])       # kl rows

            rowsum = stats.tile([P, 1], F32)
            nc.vector.reduce_sum(rowsum[:], acc[:], axis=AX)
            tot_psum = psump.tile([1, 1], F32)
            nc.tensor.matmul(tot_psum[:], lhsT=rowsum[:], rhs=ones[:],
                             start=True, stop=True)
            tot = stats.tile([1, 1], F32)
            nc.scalar.copy(tot[:], tot_psum[:])
            # per-core partial sum; host combines the 8 partials (the
            # cross-core psum via AllReduce costs ~30us of skew-wait, far
            # more than the 8-float host add)
            nc.sync.dma_start(out_d.ap(), tot[:])

    nc.compile()
    _CACHE["nc"] = nc
    return nc


def _prep_in_maps(inputs, matric, targets, label_lengths):
    x = np.ascontiguousarray(np.asarray(inputs, dtype=np.float32)).reshape(N, V)
    t = np.asarray(targets).reshape(-1).astype(np.int64)
    lab = np.asarray(label_lengths).reshape(-1).astype(np.int64)
    mat = np.ascontiguousarray(np.asarray(matric, dtype=np.float32)).reshape(V * V, 1)

    eos = (t == 1)
    prev = np.roll(t, 1)
    is_start = np.roll(eos, 1)
    is_start[0] = True
    forth = np.where(is_start, N - 1, prev)
    seg = np.cumsum(eos.astype(np.int64)) - eos.astype(np.int64)
    length = lab + 1
    # jax gather clamps out-of-range indices; mirror that
    len_row = length[np.clip(seg, 0, B - 1)].astype(np.float32)
    midx = (np.clip(forth, 0, V - 1) * V + np.clip(t, 0, V - 1)).astype(np.int32)
    t_cl = np.clip(t, 0, V - 1)
    lensum = np.float32(length.sum())

    in_maps = []
    for c in range(N_CORES):
        sl = slice(c * NLOC, (c + 1) * NLOC)
        rows = np.arange(NLOC, dtype=np.int64)
        xg = (rows * V + t_cl[sl]).astype(np.int32)
        in_maps.append({
            "x": np.ascontiguousarray(x[sl]),
            "mat": mat,
            "midx": np.ascontiguousarray(midx[sl].reshape(NT, P).T),
            "xgidx": np.ascontiguousarray(xg.reshape(NT, P).T),
            "lenrow": np.ascontiguousarray(
                len_row[sl].reshape(NT, P).T),
        })
    return in_maps, lensum


def run(inputs, matric, targets, label_lengths, trace=False):
    nc = _build()
    in_maps, lensum = _prep_in_maps(inputs, matric, targets, label_lengths)
    if trace:
        _install_ntff_hook()
    res = bass_utils.run_bass_kernel_spmd(
        nc, in_maps, core_ids=list(range(N_CORES)), trace=trace)
    partials = np.array(
        [res.results[c]["out"][0, 0] for c in range(N_CORES)], dtype=np.float32)
    out = np.float32(partials.sum(dtype=np.float32) / lensum)
    return np.asarray(out), res


def kernel(inputs, matric, targets, label_lengths):
    out, _ = run(inputs, matric, targets, label_lengths, trace=False)
    return out


def _install_ntff_hook():
    """bass_utils expects antenv.axon_hooks for NTFF tracing under axon; the
    agent image lacks it, so recreate the ctypes shim inline."""
    import contextlib
    import ctypes
    import sys
    import types

    if "antenv.axon_hooks" in sys.modules:
        return
    so_path = "/opt/axon/libaxon_pjrt.so"
    try:
        lib = ctypes.CDLL(so_path)
    except OSError:
        return
    if not hasattr(lib, "axon_start_nrt_profile"):
        return
    lib.axon_start_nrt_profile.argtypes = [
        ctypes.POINTER(ctypes.c_int64), ctypes.c_size_t]
    lib.axon_start_nrt_profile.restype = ctypes.c_int64
    lib.axon_stop_nrt_profile.argtypes = [ctypes.c_char_p]
    lib.axon_stop_nrt_profile.restype = ctypes.c_int64

    @contextlib.contextmanager
    def _hook(output_dir, device_ids):
        import jax
        jax.devices()
        ids = list(device_ids) if device_ids else []
        arr = (ctypes.c_int64 * len(ids))(*ids)
        rc = lib.axon_start_nrt_profile(arr, len(ids))
        if rc != 0:
            raise RuntimeError(f"axon_start_nrt_profile rc={rc}")
        try:
            yield
        finally:
            n = lib.axon_stop_nrt_profile(str(output_dir).encode())
            if n < 0:
                raise RuntimeError(f"axon_stop_nrt_profile rc={n}")

    mod = types.ModuleType("antenv.axon_hooks")
    mod.get_axon_ntff_profile_hook = lambda: _hook
    mod.set_axon_ntff_profile_hook = lambda h: None
    sys.modules["antenv.axon_hooks"] = mod
